# revision 34
# baseline (speedup 1.0000x reference)
"""MultiHeadAttention (B=4, S=2048, D=1024, H=16, rel-pos bias) on 8 TRN2 cores.

Sharding: core c -> batch b=c//2, head-group g=c%2 (8 heads each).
Per-core kernel computes partial out^T = Wo_g @ ctx_g^T  [1024, 2048] fp32;
host sums the two head-group partials per batch, transposes, adds bo.

Key packing: the mask is per-key (broadcast over heads/queries), so the host
gathers only the valid keys (~1024 of 2048) into nkt*128 padded slots (nkt
derived from the actual mask, same for all cores); K/V projections, scores,
exp, and ctx all shrink accordingly.  Padding slots carry a zeroed bias
strip, so their softmax weight is exactly 0.

Per-core pipeline (all matmuls fp16):
  phase 1: QKV projections, x^T streamed in small chunks (host pre-gathered
           and pre-transposed; keys packed for K/V).
  phase 2: S^T = K^T q in PSUM [128 keys, 1024 queries]; exp on ACT
           (scale 0.125, no mask bias needed); combined rel-pos bias + mask
           applied multiplicatively (es *= exp(rel_emb) gathered strip, DVE
           fp16 2x); ctx^T accumulated via [V|1] trick, denominator row 64;
           normalization = reciprocal_approx_fast + gpsimd broadcast + mul.
  phase 3: out^T = Wo_g @ ctxn, PSUM->SBUF copies on ACT, DMA out.
"""

import numpy as np
import ml_dtypes

S = 2048
D = 1024
DK = 64
B = 4
NCORES = 8
HPC = 8   # heads per core
NPAIR = 4

_CACHE = {}


def _build(nkt, cls):
    import concourse.bass as bass
    import concourse.mybir as mybir
    from concourse import bacc, tile

    f16 = mybir.dt.float16
    f32 = mybir.dt.float32
    AF = mybir.ActivationFunctionType

    nkp = 128 * nkt
    kch = [(s, min(512, nkp - s)) for s in range(0, nkp, 512)]
    mixed = {jh: [i for i in range(nkt) if cls[(i, jh)] == "m"] for jh in range(2)}
    nm_off = {0: 0, 1: len(mixed[0])}
    nm_tot = len(mixed[0]) + len(mixed[1])
    nm_max = max(len(mixed[0]), len(mixed[1]))

    nc = bacc.Bacc("TRN2", target_bir_lowering=False, debug=False,
                   num_devices=NCORES)

    def din(name, shape, dt=f16):
        return nc.dram_tensor(name, shape, dt, kind="ExternalInput").ap()

    xq_d = din("xq", [128, 8, S])
    xk_d = din("xk", [128, 8, nkp])
    xv_d = din("xv", [128, 8, nkp])
    wq_d = din("wq", [128, 8, 512])
    wk_d = din("wk", [128, 8, 512])
    wv_d = din("wv", [128, 8, 512])
    wo_d = din("wo", [128, 4, 1024])
    bq_d = din("bq", [128, 4], f32)
    bk_d = din("bk", [128, 4], f32)
    bvb_d = din("bvb", [128, 512], f32)
    str_d = din("strips", [HPC, 128, nm_tot, 1024])
    lnc_d = din("lnc", [128, 16], f32)
    out_d = nc.dram_tensor("outT", [D, S], f32, kind="ExternalOutput").ap()

    with tile.TileContext(nc) as tc:
        with (
            tc.tile_pool(name="const", bufs=1) as cpool,
            tc.tile_pool(name="qk", bufs=1) as qkpool,
            tc.tile_pool(name="vp", bufs=1) as vpool,
            tc.tile_pool(name="wo", bufs=1) as wopool,
            tc.tile_pool(name="ps", bufs=2, space="PSUM") as ps,
        ):
            bq_s = cpool.tile([128, 4], f32)
            bk_s = cpool.tile([128, 4], f32)
            bvb_s = cpool.tile([128, 512], f32)
            lnc_s = cpool.tile([128, 16], f32)
            nc.sync.dma_start(lnc_s[:], lnc_d[:])
            nc.sync.dma_start(bq_s[:], bq_d[:])
            nc.sync.dma_start(bk_s[:], bk_d[:])
            nc.sync.dma_start(bvb_s[:], bvb_d[:])

            qt = qkpool.tile([128, 4, S], f16, tag="qt")
            kt = qkpool.tile([128, 4, nkp], f16, tag="kt")
            vaug = vpool.tile([128, nkt, 520], f16)
            wo_s = wopool.tile([128, 4, 1024], f16)
            # ones columns of V_aug
            nc.vector.memset(
                vaug.rearrange("p k (h e) -> p k h e", h=8)[:, :, :, 64:65], 1.0)

            # ---- phase 1: stream x^T chunks, QKV projections ----
            with (
                tc.tile_pool(name="w", bufs=1) as wpool,
                tc.tile_pool(name="xs", bufs=5) as xspool,
                tc.tile_pool(name="xv", bufs=5) as xvpool,
            ):
                wq_s = wpool.tile([128, 8, 512], f16, tag="wq")
                wk_s = wpool.tile([128, 8, 512], f16, tag="wk")
                wv_s = wpool.tile([128, 8, 512], f16, tag="wv")
                nc.sync.dma_start(wq_s[:, 0:2, :], wq_d[:, 0:2, :])
                nc.sync.dma_start(wq_s[:, 2:8, :], wq_d[:, 2:8, :])
                nc.sync.dma_start(wk_s[:], wk_d[:])
                nc.sync.dma_start(wv_s[:], wv_d[:])

                # Q: out [pair-feat 128, seq 512] per (s4, p)
                for s4 in range(4):
                    xq4 = xspool.tile([128, 8, 512], f16, tag="xs")
                    if s4 == 0:
                        nc.sync.dma_start(xq4[:, 0:2, :], xq_d[:, 0:2, 0:512])
                        nc.sync.dma_start(xq4[:, 2:8, :], xq_d[:, 2:8, 0:512])
                    else:
                        nc.sync.dma_start(xq4[:], xq_d[:, :, s4 * 512:(s4 + 1) * 512])
                    for p in range(NPAIR):
                        pt = ps.tile([128, 1024], f32, tag="ps")
                        acc = pt[:, 0:512]
                        for c in range(8):
                            nc.tensor.matmul(
                                acc, wq_s[:, c, p * 128:(p + 1) * 128],
                                xq4[:, c, :], start=(c == 0), stop=(c == 7))
                        nc.vector.tensor_scalar_add(
                            qt[:, p, s4 * 512:(s4 + 1) * 512], acc, bq_s[:, p:p + 1])
                # K over packed keys
                for (s0, sz) in kch:
                    xk4 = xspool.tile([128, 8, 512], f16, tag="xs")
                    nc.sync.dma_start(xk4[:, :, 0:sz], xk_d[:, :, s0:s0 + sz])
                    for p in range(NPAIR):
                        pt = ps.tile([128, 1024], f32, tag="ps")
                        acc = pt[:, 0:sz]
                        for c in range(8):
                            nc.tensor.matmul(
                                acc, wk_s[:, c, p * 128:(p + 1) * 128],
                                xk4[:, c, 0:sz], start=(c == 0), stop=(c == 7))
                        nc.vector.tensor_scalar_add(
                            kt[:, p, s0:s0 + sz], acc, bk_s[:, p:p + 1])
                # V: out [keys 128, dv 512] per packed key-tile
                for i in range(nkt):
                    xvi = xvpool.tile([128, 8, 128], f16, tag="xv")
                    nc.sync.dma_start(xvi[:], xv_d[:, :, i * 128:(i + 1) * 128])
                    pt = ps.tile([128, 1024], f32, tag="ps")
                    acc = pt[:, 0:512]
                    for c in range(8):
                        nc.tensor.matmul(
                            acc, xvi[:, c, :],
                            wv_s[:, c, :], start=(c == 0), stop=(c == 7))
                    nc.vector.tensor_add(
                        vaug[:, i, :].rearrange("p (h e) -> p h e", h=8)[:, :, 0:64],
                        acc.rearrange("p (h e) -> p h e", e=64), bvb_s.rearrange("p (h e) -> p h e", e=64))
                nc.sync.dma_start(wo_s[:], wo_d[:])

            # ---- phase 2: attention ----
            with (
                tc.tile_pool(name="strips", bufs=2) as spool,
                tc.tile_pool(name="es", bufs=10) as espool,
                tc.tile_pool(name="ctxn", bufs=1) as cnpool,
                tc.tile_pool(name="rc", bufs=2) as rcpool,
                tc.tile_pool(name="cx", bufs=2, space="PSUM") as cx,
                tc.tile_pool(name="oev", bufs=2) as oevpool,
            ):
                ctxn = cnpool.tile([128, 4, S], f16)
                for p in range(NPAIR):
                    for jh in range(2):
                        nm = len(mixed[jh])
                        strip = spool.tile([128, 2, nm_max, 1024], f16, tag="strip")
                        for e in range(2):
                            nc.sync.dma_start(
                                strip[:, e, 0:nm, :],
                                str_d[2 * p + e, :, nm_off[jh]:nm_off[jh] + nm, :])
                        cxt = [cx.tile([65, 1024], f32, tag="cx", name=f"cxt{_e}")
                               for _e in range(2)]
                        mi = 0
                        for i in range(nkt):
                            kind = cls[(i, jh)]
                            for e in range(2):
                                h = 2 * p + e
                                st = ps.tile([128, 1024], f32, tag="ps")
                                for jq in range(2):
                                    q0 = (2 * jh + jq) * 512
                                    sl = st[:, jq * 512:(jq + 1) * 512]
                                    nc.tensor.matmul(
                                        sl, kt[64 * e:64 * e + 64, p, i * 128:(i + 1) * 128],
                                        qt[64 * e:64 * e + 64, p, q0:q0 + 512],
                                        start=True, stop=True)
                                es = espool.tile([128, 1024], f16, tag="es")
                                if kind == "m":
                                    er = espool.tile([128, 1024], f16, tag="er")
                                    nc.scalar.activation(er[:], st[:], AF.Exp,
                                                         scale=0.125)
                                    nc.vector.tensor_mul(
                                        es[:], er[:], strip[:, e, mi, :])
                                else:
                                    col = (0 if kind == "p" else 8) + h
                                    nc.scalar.activation(
                                        es[:], st[:], AF.Exp,
                                        bias=lnc_s[:, col:col + 1], scale=0.125)
                                for jq in range(2):
                                    nc.tensor.matmul(
                                        cxt[e][:, jq * 512:(jq + 1) * 512],
                                        vaug[:, i, 65 * h:65 * h + 65],
                                        es[:, jq * 512:(jq + 1) * 512],
                                        start=(i == 0), stop=(i == nkt - 1))
                            if kind == "m":
                                mi += 1
                        for e in range(2):
                            den = rcpool.tile([1, 1024], f32, tag="den")
                            rcp = rcpool.tile([1, 1024], f32, tag="rcp")
                            rcb = rcpool.tile([64, 1024], f32, tag="rcb")
                            nc.vector.tensor_copy(den[:], cxt[e][64:65, :])
                            nc.vector.reciprocal_approx_fast(
                                out=rcp[:], in_=den[:])
                            nc.gpsimd.partition_broadcast(rcb[:], rcp[:])
                            nc.vector.tensor_mul(
                                ctxn[64 * e:64 * e + 64, p, jh * 1024:(jh + 1) * 1024],
                                cxt[e][0:64, :], rcb[:])

                # ---- phase 3: output projection: outT [1024, 2048] ----
                for d in range(8):
                    oev = oevpool.tile([128, 2048], f32, tag="oev")
                    for jq in range(4):
                        pt = ps.tile([128, 1024], f32, tag="ps")
                        acc = pt[:, 0:512]
                        for c in range(4):
                            nc.tensor.matmul(
                                acc, wo_s[:, c, d * 128:(d + 1) * 128],
                                ctxn[:, c, jq * 512:(jq + 1) * 512],
                                start=(c == 0), stop=(c == 3))
                        nc.scalar.copy(oev[:, jq * 512:(jq + 1) * 512], acc)
                        if jq == 1:
                            nc.sync.dma_start(
                                out_d[d * 128:(d + 1) * 128, 0:1024],
                                oev[:, 0:1024])
                    nc.sync.dma_start(
                        out_d[d * 128:(d + 1) * 128, 1024:2048],
                        oev[:, 1024:2048])

    nc.compile()
    return nc


def _classify(mask, nkt):
    nkp = 128 * nkt
    cls = {}
    pmin = np.full(nkt, 10 ** 9)
    pmax = np.full(nkt, -1)
    min_nv = nkp
    for b in range(B):
        valid = np.where(np.asarray(mask[b, 0, 0, :]) != 0)[0]
        nv = len(valid)
        min_nv = min(min_nv, nv)
        for i in range(nkt):
            lo, hi = i * 128, min((i + 1) * 128, nv)
            if hi > lo:
                pmin[i] = min(pmin[i], valid[lo])
                pmax[i] = max(pmax[i], valid[hi - 1])
    for i in range(nkt):
        for jh in range(2):
            full = (i + 1) * 128 <= min_nv
            if full and jh * 1024 + 1023 < pmin[i] - 128:
                cls[(i, jh)] = "p"
            elif full and jh * 1024 > pmax[i] + 128:
                cls[(i, jh)] = "n"
            else:
                cls[(i, jh)] = "m"
    return cls


def _host_inputs(query, key, value, mask, Wq, bq, Wk, bk, Wv, bv, Wo, bo, rel_emb,
                 nkt, cls):
    f16 = np.float16
    nkp = 128 * nkt

    def tform(xT, n):
        # x^T [D, n] -> [128, 8, n]
        return np.ascontiguousarray(
            xT.reshape(8, 128, n).transpose(1, 0, 2)).astype(f16)

    def wform(W, sl):
        return np.ascontiguousarray(
            np.asarray(W).T[:, sl].reshape(8, 128, 512).transpose(1, 0, 2)).astype(f16)

    es_tab = np.exp(np.asarray(rel_emb).astype(np.float32))  # [257, 16]
    pos = np.arange(S)
    mixed = {jh: [i for i in range(nkt) if cls[(i, jh)] == "m"] for jh in range(2)}
    nm_tot = len(mixed[0]) + len(mixed[1])

    batch_prep = []
    for b in range(B):
        m = np.asarray(mask[b, 0, 0, :])
        valid = np.where(m != 0)[0]
        nv = len(valid)
        assert nv <= nkp, f"too many valid keys: {nv}"
        xk_g = np.zeros((nkp, D), np.float32)
        xv_g = np.zeros((nkp, D), np.float32)
        xk_g[:nv] = np.asarray(key[b])[valid]
        xv_g[:nv] = np.asarray(value[b])[valid]
        pos_pad = np.zeros(nkp, np.int64)
        pos_pad[:nv] = valid
        ridx = np.clip(pos_pad[:, None] - pos[None, :], -128, 128) + 128
        batch_prep.append((xk_g, xv_g, ridx, nv))

    in_maps = []
    for c in range(NCORES):
        b, g = divmod(c, 2)
        sl = slice(512 * g, 512 * (g + 1))
        xk_g, xv_g, ridx, nv = batch_prep[b]
        strips = np.empty((HPC, 128, nm_tot, 1024), f16)
        for hl in range(HPC):
            tab = es_tab[:, 8 * g + hl].astype(f16)
            sh = tab[ridx]                     # [nkp, S]
            sh[nv:] = 0
            sh = sh.reshape(nkt, 128, S).transpose(1, 0, 2)  # [128, nkt, S]
            m = 0
            for jh in range(2):
                for i in mixed[jh]:
                    strips[hl, :, m, :] = sh[:, i, jh * 1024:(jh + 1) * 1024]
                    m += 1
        lnc = np.tile(np.concatenate([
            np.asarray(rel_emb)[256, 8 * g:8 * g + 8],
            np.asarray(rel_emb)[0, 8 * g:8 * g + 8]]).astype(np.float32), (128, 1))
        in_maps.append({
            "xq": tform(np.asarray(query[b]).T, S),
            "xk": tform(xk_g.T, nkp),
            "xv": tform(xv_g.T, nkp),
            "wq": wform(Wq, sl),
            "wk": wform(Wk, sl),
            "wv": wform(Wv, sl),
            "wo": np.ascontiguousarray(
                np.asarray(Wo).T[sl, :].reshape(4, 128, 1024).transpose(1, 0, 2)).astype(f16),
            "bq": np.ascontiguousarray(np.asarray(bq)[sl].reshape(4, 128).T).astype(np.float32),
            "bk": np.ascontiguousarray(np.asarray(bk)[sl].reshape(4, 128).T).astype(np.float32),
            "bvb": np.tile(np.asarray(bv)[sl].astype(np.float32), (128, 1)),
            "strips": strips,
            "lnc": np.ascontiguousarray(lnc),
        })
    return in_maps


def kernel(query, key, value, mask, Wq, bq, Wk, bk, Wv, bv, Wo, bo, rel_emb,
           _trace=False, _trace_kwargs=None):
    from concourse import bass_utils
    nkt = 0
    for b in range(B):
        nv = int(np.count_nonzero(np.asarray(mask[b, 0, 0, :])))
        nkt = max(nkt, -(-nv // 128))
    cls = _classify(mask, nkt)
    key_ = ("nc", nkt, tuple(sorted(cls.items())))
    if key_ not in _CACHE:
        _CACHE[key_] = _build(nkt, cls)
    nc = _CACHE[key_]
    in_maps = _host_inputs(query, key, value, mask, Wq, bq, Wk, bk, Wv, bv,
                           Wo, bo, rel_emb, nkt, cls)
    res = bass_utils.run_bass_kernel_spmd(
        nc, in_maps, core_ids=list(range(NCORES)), trace=_trace,
        **(_trace_kwargs or {}))
    _CACHE["last_res"] = res
    out = np.zeros((B, S, D), np.float32)
    for b in range(B):
        acc = res.results[2 * b]["outT"] + res.results[2 * b + 1]["outT"]
        out[b] = acc.T
    out += np.asarray(bo).astype(np.float32)[None, None, :]
    return out
